# revision 1
# baseline (speedup 1.0000x reference)
"""Causal self-attention (RoPE) Trainium2 Bass kernel.

Sharding: 8 cores = 4 batches x 2 head-groups. Core c handles batch c//2 and
heads (c%2)*8 .. (c%2)*8+7. Each core computes its QKV projection slice, RoPE,
causal flash-style attention in transposed layout, and a partial output
projection; the host sums the two partial projections per batch.

All matmuls use float32r (TF32-like, ~1.5e-4 rel err) at full PE rate.
Attention is computed transposed (s^T = k q^T) so softmax denominators come
from an appended ones-column in the value matrix and attention output feeds
the output projection as lhsT with no transposes.
"""

import math
import numpy as np
from contextlib import ExitStack

import concourse.bass as bass
import concourse.tile as tile
from concourse import bacc, mybir
from concourse.bass_utils import run_bass_kernel_spmd

F32 = mybir.dt.float32
R32 = mybir.dt.float32r
EXPF = mybir.ActivationFunctionType.Exp
MULT = mybir.AluOpType.mult
ADD = mybir.AluOpType.add

B, T, C, H, D = 4, 2048, 1024, 16, 64
HL = 8            # local heads per core
NP = HL // 2      # head pairs per core
KT = C // 128     # contraction tiles for projections
TT = T // 128     # 128-row tiles of T
QC = T // 512     # 512-col chunks of T
SCALE = 1.0 / math.sqrt(D)

_CACHE = {}


def _build_nc():
    nc = bacc.Bacc("TRN2", debug=False, num_devices=8)

    xT_d = nc.dram_tensor("xT", [KT, 128, T], R32, kind="ExternalInput").ap()
    wq_d = nc.dram_tensor("wq", [128, NP, KT, 128], R32, kind="ExternalInput").ap()
    wk_d = nc.dram_tensor("wk", [128, NP, KT, 128], R32, kind="ExternalInput").ap()
    wv_d = nc.dram_tensor("wv", [128, KT, 512], R32, kind="ExternalInput").ap()
    wo_d = nc.dram_tensor("wo", [128, NP, C], R32, kind="ExternalInput").ap()
    cos_d = nc.dram_tensor("cosT", [128, T], F32, kind="ExternalInput").ap()
    sin_d = nc.dram_tensor("sinT", [128, T], F32, kind="ExternalInput").ap()
    psw_d = nc.dram_tensor("psw", [128, 128], R32, kind="ExternalInput").ap()
    e64_d = nc.dram_tensor("e64", [128, 64], R32, kind="ExternalInput").ap()
    msk_d = nc.dram_tensor("msk", [128, 4, 512], F32, kind="ExternalInput").ap()
    wrm_d = nc.dram_tensor("wrm", [128, 512], R32, kind="ExternalInput").ap()
    out_d = nc.dram_tensor("out", [T, C], F32, kind="ExternalOutput").ap()

    with tile.TileContext(nc) as tc:
        with ExitStack() as ctx:
            pers = ctx.enter_context(tc.tile_pool(name="pers", bufs=1))
            vext = pers.tile([128, TT, HL, D + 1], R32)
            qkT = {}
            for p in range(NP):
                for s in "qk":
                    qkT[(p, s)] = pers.tile([128, T], R32, name=f"qkT_{p}_{s}")
            yT = [pers.tile([128, T], R32, name=f"yT_{r}") for r in range(NP)]
            nc.gpsimd.memset(vext[:, :, :, D].bitcast(F32), 1.0)

            # ---- phase V: value projection -> vext (natural layout + ones col)
            with (
                tc.tile_pool(name="vph", bufs=2) as vp,
                tc.tile_pool(name="vw", bufs=1) as vw,
                tc.tile_pool(name="vps", bufs=2, space="PSUM") as vps,
            ):
                wv_sb = vw.tile([128, KT, 512], R32)
                nc.sync.dma_start(wv_sb[:], wv_d)
                for tt in range(TT):
                    xv = vp.tile([128, KT, 128], R32, tag="xv")
                    nc.sync.dma_start(
                        xv[:],
                        xT_d[:, :, tt * 128 : (tt + 1) * 128].rearrange(
                            "k c t -> c k t"
                        ),
                    )
                    ps = vps.tile([128, 512], F32, tag="pv")
                    for kt in range(KT):
                        nc.tensor.matmul(
                            ps[:], xv[:, kt], wv_sb[:, kt],
                            start=(kt == 0), stop=(kt == KT - 1),
                        )
                    nc.vector.tensor_copy(vext[:, tt, :, 0:D], ps[:])

            # ---- phase QK: q/k projection + RoPE -> qkT (transposed layout)
            with (
                tc.tile_pool(name="qkst", bufs=2) as sp,
                tc.tile_pool(name="qkw", bufs=1) as qw,
                tc.tile_pool(name="ctab", bufs=2) as ct,
                tc.tile_pool(name="qkps", bufs=2, space="PSUM") as qps,
                tc.tile_pool(name="rotps", bufs=2, space="PSUM") as rps,
            ):
                wq_sb = qw.tile([128, NP, KT, 128], R32)
                wk_sb = qw.tile([128, NP, KT, 128], R32)
                psw_sb = qw.tile([128, 128], R32)
                nc.sync.dma_start(wq_sb[:], wq_d)
                nc.sync.dma_start(wk_sb[:], wk_d)
                nc.sync.dma_start(psw_sb[:], psw_d)
                for qc in range(QC):
                    lo, hi = qc * 512, (qc + 1) * 512
                    xc = sp.tile([128, KT, 512], R32, tag="xc")
                    nc.sync.dma_start(
                        xc[:], xT_d[:, :, lo:hi].rearrange("k c t -> c k t")
                    )
                    cosc = ct.tile([128, 512], F32, tag="cosc")
                    sinc = ct.tile([128, 512], F32, tag="sinc")
                    nc.sync.dma_start(cosc[:], cos_d[:, lo:hi])
                    nc.sync.dma_start(sinc[:], sin_d[:, lo:hi])
                    for p in range(NP):
                        for w_sb, key in ((wq_sb, "q"), (wk_sb, "k")):
                            dst = qkT[(p, key)][:, lo:hi]
                            ps = qps.tile([128, 512], F32, tag="pq")
                            for kt in range(KT):
                                nc.tensor.matmul(
                                    ps[:], w_sb[:, p, kt], xc[:, kt],
                                    start=(kt == 0), stop=(kt == KT - 1),
                                )
                            nc.vector.tensor_tensor(
                                dst, ps[:], cosc[:], MULT
                            )
                            u = sp.tile([128, 512], R32, tag="u")
                            nc.vector.tensor_tensor(
                                u[:], ps[:], sinc[:], MULT
                            )
                            pr = rps.tile([128, 512], F32, tag="pr")
                            nc.tensor.matmul(
                                pr[:], psw_sb[:], u[:], start=True, stop=True
                            )
                            nc.vector.tensor_tensor(dst, pr[:], dst, ADD)

            # ---- phase ATT: causal attention per head pair, transposed
            with (
                tc.tile_pool(name="attp", bufs=3) as ap_,
                tc.tile_pool(name="atab", bufs=1) as at_,
                tc.tile_pool(name="sps", bufs=2, space="PSUM") as sps,
                tc.tile_pool(name="yps", bufs=2, space="PSUM") as yps,
            ):
                e64_sb = at_.tile([128, 64], R32)
                msk_sb = at_.tile([128, 4, 512], F32)
                rrec = at_.tile([128, 512], R32)
                rscr = at_.tile([128, 512], F32)
                rscr2 = at_.tile([128, 512], F32)
                ww_sb = at_.tile([128, 128], R32)
                wrm_sb = at_.tile([128, 512], R32)
                nc.sync.dma_start(e64_sb[:], e64_d)
                nc.sync.dma_start(msk_sb[:], msk_d)
                nc.sync.dma_start(ww_sb[:], psw_d)
                nc.sync.dma_start(wrm_sb[:], wrm_d)
                nc.gpsimd.memset(rrec[:].bitcast(F32), 0.0)
                for p in range(NP):
                    qTt = qkT[(p, "q")]
                    kTt = qkT[(p, "k")]
                    for qc in range(QC):
                        lo, hi = qc * 512, (qc + 1) * 512
                        nkt = (qc + 1) * 4
                        psyA = yps.tile([65, 512], F32, tag="yA")
                        psyB = yps.tile([65, 512], F32, tag="yB")
                        for kt in range(nkt):
                            first, last = kt == 0, kt == nkt - 1
                            klo, khi = kt * 128, (kt + 1) * 128
                            off = klo - lo
                            # valid q range for this tile is [off, 512); trim
                            # to it when the fp32r fast path allows (>=256)
                            tr = off if off in (128, 256) else 0
                            w = 512 - tr
                            ps2 = sps.tile([128, 1024], F32, tag="sA")
                            p3 = ps2[:].rearrange("p (h n) -> p h n", h=2)
                            nc.tensor.matmul(
                                ps2[:, tr:512],
                                kTt[0:64, klo:khi], qTt[0:64, lo + tr : hi],
                                start=True, stop=True,
                            )
                            nc.tensor.matmul(
                                ps2[:, 512 + tr : 1024],
                                kTt[64:128, klo:khi], qTt[64:128, lo + tr : hi],
                                start=True, stop=True,
                            )
                            aAB = ap_.tile([128, 1024], R32, tag="aA")
                            a3 = aAB[:].rearrange("p (h n) -> p h n", h=2)
                            aA = aAB[:, 0:512]
                            aB = aAB[:, 512:1024]
                            nc.scalar.activation(
                                a3[:, :, tr:512], p3[:, :, tr:512],
                                EXPF, scale=SCALE,
                            )
                            if off >= 0:
                                mi = off // 128
                                nc.vector.tensor_tensor(
                                    aA[:, tr:512], aA[:, tr:512],
                                    msk_sb[:, mi, tr:512], MULT,
                                )
                                nc.vector.tensor_tensor(
                                    aB[:, tr:512], aB[:, tr:512],
                                    msk_sb[:, mi, tr:512], MULT,
                                )
                            nc.tensor.matmul(
                                psyA[:, tr:512], vext[:, kt, 2 * p, :],
                                aA[:, tr:512], start=first, stop=last,
                            )
                            nc.tensor.matmul(
                                psyB[:, tr:512], vext[:, kt, 2 * p + 1, :],
                                aB[:, tr:512], start=first, stop=last,
                            )
                        # normalize: recip of denom row, broadcast via selector
                        # matmul, multiply into yT
                        for hh, psy in ((0, psyA), (1, psyB)):
                            with nc.allow_low_precision(
                                reason="recip row feeds fp32r selector matmul"
                            ):
                                nc.vector.reciprocal(
                                    rrec[64:65, :], psy[64:65, :]
                                )
                            pbc = sps.tile([64, 512], F32, tag="sA", name="pbc")
                            nc.tensor.matmul(
                                pbc[:], e64_sb[:], rrec[:], start=True, stop=True
                            )
                            bc = ap_.tile([64, 512], R32, tag="bc")
                            nc.scalar.copy(bc[:], pbc[:])
                            if hh == 0:
                                nc.vector.tensor_tensor(
                                    yT[p][0:64, lo:hi], psy[0:64, :], bc[:], MULT
                                )
                            else:
                                tb = ap_.tile([64, 512], R32, tag="tb")
                                nc.vector.tensor_tensor(
                                    tb[:], psy[0:64, :], bc[:], MULT
                                )
                                nc.sync.dma_start(yT[p][64:128, lo:hi], tb[:])

            # ---- phase OUT: output projection (partial; host sums over cores)
            with (
                tc.tile_pool(name="oph", bufs=3) as op_,
                tc.tile_pool(name="ow", bufs=1) as ow,
                tc.tile_pool(name="ops", bufs=4, space="PSUM") as ops,
            ):
                wo_sb = ow.tile([128, NP, C], R32)
                nc.sync.dma_start(wo_sb[:], wo_d)
                for mt in range(TT):
                    mlo, mhi = mt * 128, (mt + 1) * 128
                    for cc in range(2):
                        clo, chi = cc * 512, (cc + 1) * 512
                        ps = ops.tile([128, 512], F32, tag="po")
                        for r in range(NP):
                            nc.tensor.matmul(
                                ps[:], yT[r][:, mlo:mhi], wo_sb[:, r, clo:chi],
                                start=(r == 0), stop=(r == NP - 1),
                            )
                        ob = op_.tile([128, 512], F32, tag="ob")
                        nc.vector.tensor_copy(ob[:], ps[:])
                        nc.sync.dma_start(out_d[mlo:mhi, clo:chi], ob[:])

    nc.compile()
    return nc


def _host_tables():
    half = D // 2
    freq = np.exp(-math.log(10000.0) * np.arange(half) / half).astype(np.float64)
    ang = np.arange(T, dtype=np.float64)[None, :] * freq[:, None]  # [32, T]
    cos32 = np.cos(ang).astype(np.float32)
    sin32 = np.sin(ang).astype(np.float32)
    cosT = np.tile(cos32, (4, 1))                                   # [128, T]
    sinT = np.concatenate([sin32, -sin32, sin32, -sin32], axis=0)   # [128, T]
    psw = np.zeros((128, 128), np.float32)
    psw[np.arange(128) ^ 32, np.arange(128)] = 1.0
    e64 = np.zeros((128, 64), np.float32)
    e64[64, :] = 1.0
    kk = np.arange(128)[:, None, None]
    ii = np.arange(4)[None, :, None]
    qq = np.arange(512)[None, None, :]
    msk = (qq >= kk + ii * 128).astype(np.float32)
    return cosT, sinT, psw, e64, msk


def _pack_weights(w_qkv, w_out, hg):
    lo, hi = hg * HL, (hg + 1) * HL
    wqf = w_qkv[:, 0:C].reshape(C, H, D)[:, lo:hi]       # [C, 8, D]
    wkf = w_qkv[:, C : 2 * C].reshape(C, H, D)[:, lo:hi]
    wvf = w_qkv[:, 2 * C : 3 * C].reshape(C, H, D)[:, lo:hi]

    def pack_qk(w):
        a = w.reshape(KT, 128, NP, 2, D)
        return np.ascontiguousarray(
            a.transpose(1, 2, 0, 3, 4).reshape(128, NP, KT, 128)
        )

    wq = pack_qk(wqf)
    wk = pack_qk(wkf)
    wv = np.ascontiguousarray(
        wvf.reshape(KT, 128, HL * D).transpose(1, 0, 2)
    )
    wo_l = w_out.reshape(H, D, C)[lo:hi].reshape(NP, 128, C)
    wo = np.ascontiguousarray(wo_l.transpose(1, 0, 2))
    return wq, wk, wv, wo


def kernel(x, w_qkv, w_out):
    x = np.asarray(x, dtype=np.float32)
    w_qkv = np.asarray(w_qkv, dtype=np.float32)
    w_out = np.asarray(w_out, dtype=np.float32)

    if "nc" not in _CACHE:
        _CACHE["nc"] = _build_nc()
    nc = _CACHE["nc"]

    cosT, sinT, psw, e64, msk = _host_tables()
    packs = [_pack_weights(w_qkv, w_out, hg) for hg in range(2)]
    xTs = [
        np.ascontiguousarray(x[b].T).reshape(KT, 128, T) for b in range(B)
    ]

    in_maps = []
    for c in range(8):
        b, hg = c // 2, c % 2
        wq, wk, wv, wo = packs[hg]
        in_maps.append(
            {
                "xT": xTs[b], "wq": wq, "wk": wk, "wv": wv, "wo": wo,
                "cosT": cosT, "sinT": sinT, "psw": psw, "e64": e64,
                "msk": msk, "wrm": np.full((128, 512), 0.03, np.float32),
            }
        )

    res = run_bass_kernel_spmd(nc, in_maps, core_ids=list(range(8)))
    outs = [res.results[c]["out"] for c in range(8)]
    y = np.stack([outs[2 * b] + outs[2 * b + 1] for b in range(B)], axis=0)
    return y.astype(np.float32)



# revision 5
# speedup vs baseline: 1.3261x; 1.3261x over previous
"""Causal self-attention (RoPE) Trainium2 Bass kernel.

Sharding: 8 cores = 4 batches x 2 head-groups. Core c handles batch c//2 and
heads (c%2)*8 .. (c%2)*8+7. Each core computes its QKV projection slice, RoPE,
causal flash-style attention in transposed layout, and a partial output
projection; the host sums the two partial projections per batch.

All matmuls use float32r (TF32-like, ~1.5e-4 rel err) at full PE rate.
Attention is computed transposed (s^T = k q^T) so softmax denominators come
from an appended ones-column in the value matrix and attention output feeds
the output projection as lhsT with no transposes.

x is staged in DRAM as [128, KT, T] (contraction-block-major) so chunk loads
are 2KB-per-descriptor DMAs; the value projection consumes the same chunk
loads as the q/k projection. Causal tiles are trimmed to their exact valid
width, masking touches only the 128-wide diagonal band, and softmax
normalization uses the fast approximate reciprocal.
"""

import math
import numpy as np
from contextlib import ExitStack

import concourse.bass as bass
import concourse.tile as tile
from concourse import bacc, mybir
from concourse.bass_utils import run_bass_kernel_spmd

F32 = mybir.dt.float32
R32 = mybir.dt.float32r
BF16 = mybir.dt.bfloat16
EXPF = mybir.ActivationFunctionType.Exp
LNF = mybir.ActivationFunctionType.Ln
MULT = mybir.AluOpType.mult
ADD = mybir.AluOpType.add

B, T, C, H, D = 4, 2048, 1024, 16, 64
HL = 8            # local heads per core
NP = HL // 2      # head pairs per core
KT = C // 128     # contraction tiles for projections
TT = T // 128     # 128-row tiles of T
QC = T // 512     # 512-col chunks of T
SCALE = 1.0 / math.sqrt(D)

_CACHE = {}


def _build_nc():
    nc = bacc.Bacc("TRN2", debug=False, num_devices=8)

    xT_d = nc.dram_tensor("xT", [128, KT, T], R32, kind="ExternalInput").ap()
    wq_d = nc.dram_tensor("wq", [128, NP, KT, 128], R32, kind="ExternalInput").ap()
    wk_d = nc.dram_tensor("wk", [128, NP, KT, 128], R32, kind="ExternalInput").ap()
    wv_d = nc.dram_tensor("wv", [128, KT, 512], R32, kind="ExternalInput").ap()
    wo_d = nc.dram_tensor("wo", [128, NP, C], R32, kind="ExternalInput").ap()
    cos_d = nc.dram_tensor("cosT", [128, T], F32, kind="ExternalInput").ap()
    sin_d = nc.dram_tensor("sinT", [128, T], F32, kind="ExternalInput").ap()
    psw_d = nc.dram_tensor("psw", [128, 128], R32, kind="ExternalInput").ap()
    e64_d = nc.dram_tensor("e64", [128, 64], BF16, kind="ExternalInput").ap()
    msk_d = nc.dram_tensor("msk", [128, 128], F32, kind="ExternalInput").ap()
    out_d = nc.dram_tensor("out", [T, C], F32, kind="ExternalOutput").ap()

    with tile.TileContext(nc) as tc:
        with ExitStack() as ctx:
            pers = ctx.enter_context(tc.tile_pool(name="pers", bufs=1))
            vext = pers.tile([128, TT, HL, D + 1], R32)
            qkT = {}
            for p in range(NP):
                for s in "qk":
                    qkT[(p, s)] = pers.tile([128, T], BF16, name=f"qkT_{p}_{s}")
            yT = [pers.tile([128, T], R32, name=f"yT_{r}") for r in range(NP)]
            nc.gpsimd.memset(vext[:, :, :, D].bitcast(F32), 1.0)

            # ---- phase VQK: per 512-col chunk: value projection (natural
            # layout + ones col) and q/k projection + RoPE (transposed layout)
            with (
                tc.tile_pool(name="qkst", bufs=2) as sp,
                tc.tile_pool(name="qkw", bufs=1) as qw,
                tc.tile_pool(name="ctab", bufs=2) as ct,
                tc.tile_pool(name="vps", bufs=2, space="PSUM") as vps,
                tc.tile_pool(name="qkps", bufs=2, space="PSUM") as qps,
                tc.tile_pool(name="rotps", bufs=2, space="PSUM") as rps,
            ):
                wq_sb = qw.tile([128, NP, KT, 128], R32)
                wk_sb = qw.tile([128, NP, KT, 128], R32)
                wv_sb = qw.tile([128, KT, 512], R32)
                psw_sb = qw.tile([128, 128], R32)
                nc.sync.dma_start(wq_sb[:], wq_d)
                nc.sync.dma_start(wk_sb[:], wk_d)
                nc.sync.dma_start(wv_sb[:], wv_d)
                nc.sync.dma_start(psw_sb[:], psw_d)
                for qc in range(QC):
                    lo, hi = qc * 512, (qc + 1) * 512
                    xc = sp.tile([128, KT, 512], R32, tag="xc")
                    nc.sync.dma_start(xc[:], xT_d[:, :, lo:hi])
                    cosc = ct.tile([128, 512], F32, tag="cosc")
                    sinc = ct.tile([128, 512], F32, tag="sinc")
                    nc.sync.dma_start(cosc[:], cos_d[:, lo:hi])
                    nc.sync.dma_start(sinc[:], sin_d[:, lo:hi])
                    # value projection for the 4 row-tiles of this chunk;
                    # copy runs on the otherwise-idle scalar engine
                    for ts in range(4):
                        tt = qc * 4 + ts
                        pv = vps.tile([128, 512], F32, tag="pv")
                        for kt in range(KT):
                            nc.tensor.matmul(
                                pv[:],
                                xc[:, kt, ts * 128 : (ts + 1) * 128],
                                wv_sb[:, kt],
                                start=(kt == 0), stop=(kt == KT - 1),
                            )
                        nc.scalar.copy(vext[:, tt, :, 0:D], pv[:])
                    for p in range(NP):
                        for w_sb, key in ((wq_sb, "q"), (wk_sb, "k")):
                            dst = qkT[(p, key)][:, lo:hi]
                            ps = qps.tile([128, 512], F32, tag="pq")
                            for kt in range(KT):
                                nc.tensor.matmul(
                                    ps[:], w_sb[:, p, kt], xc[:, kt],
                                    start=(kt == 0), stop=(kt == KT - 1),
                                )
                            nc.vector.tensor_tensor(
                                dst, ps[:], cosc[:], MULT
                            )
                            u = sp.tile([128, 512], R32, tag="u")
                            nc.vector.tensor_tensor(
                                u[:], ps[:], sinc[:], MULT
                            )
                            pr = rps.tile([128, 512], F32, tag="pr")
                            nc.tensor.matmul(
                                pr[:], psw_sb[:], u[:], start=True, stop=True
                            )
                            nc.vector.tensor_tensor(dst, pr[:], dst, ADD)

            # ---- phase ATT: causal attention per head pair, transposed
            with (
                tc.tile_pool(name="attp", bufs=4) as ap_,
                tc.tile_pool(name="atab", bufs=1) as at_,
                tc.tile_pool(name="sps", bufs=2, space="PSUM") as sps,
                tc.tile_pool(name="yps", bufs=2, space="PSUM") as yps,
            ):
                e64_sb = at_.tile([128, 64], BF16)
                msk_sb = at_.tile([128, 128], F32)
                rrecF = [
                    at_.tile([128, 512], F32, name=f"rrecF{i}") for i in range(2)
                ]
                rrec16 = [
                    at_.tile([128, 512], BF16, name=f"rrec16_{i}")
                    for i in range(2)
                ]
                nc.sync.dma_start(e64_sb[:], e64_d)
                nc.sync.dma_start(msk_sb[:], msk_d)
                for rr in rrec16:
                    nc.gpsimd.memset(rr[:], 0.0)
                for p in range(NP):
                    qTt = qkT[(p, "q")]
                    kTt = qkT[(p, "k")]
                    for qc in range(QC):
                        lo, hi = qc * 512, (qc + 1) * 512
                        nkt = (qc + 1) * 4
                        psyA = yps.tile([65, 512], F32, tag="yA")
                        psyB = yps.tile([65, 512], F32, tag="yB")
                        for kt in range(nkt):
                            first, last = kt == 0, kt == nkt - 1
                            klo, khi = kt * 128, (kt + 1) * 128
                            off = klo - lo
                            # trim to the exact causally-valid query range
                            tr = max(off, 0)
                            ps2 = sps.tile([128, 1024], F32, tag="sA")
                            p3 = ps2[:].rearrange("p (h n) -> p h n", h=2)
                            nc.tensor.matmul(
                                ps2[:, tr:512],
                                kTt[0:64, klo:khi], qTt[0:64, lo + tr : hi],
                                start=True, stop=True,
                            )
                            nc.tensor.matmul(
                                ps2[:, 512 + tr : 1024],
                                kTt[64:128, klo:khi], qTt[64:128, lo + tr : hi],
                                start=True, stop=True,
                            )
                            aAB = ap_.tile([128, 1024], R32, tag="aA")
                            a3 = aAB[:].rearrange("p (h n) -> p h n", h=2)
                            aA = aAB[:, 0:512]
                            aB = aAB[:, 512:1024]
                            nc.scalar.activation(
                                a3[:, :, tr:512], p3[:, :, tr:512],
                                EXPF, scale=SCALE,
                            )
                            if off >= 0:
                                # only the 128-wide diagonal band needs the
                                # triangular mask; columns past it are fully
                                # visible
                                nc.vector.tensor_tensor(
                                    aA[:, tr : tr + 128], aA[:, tr : tr + 128],
                                    msk_sb[:], MULT,
                                )
                                nc.vector.tensor_tensor(
                                    aB[:, tr : tr + 128], aB[:, tr : tr + 128],
                                    msk_sb[:], MULT,
                                )
                            nc.tensor.matmul(
                                psyA[:, tr:512], vext[:, kt, 2 * p, :],
                                aA[:, tr:512], start=first, stop=last,
                            )
                            nc.tensor.matmul(
                                psyB[:, tr:512], vext[:, kt, 2 * p + 1, :],
                                aB[:, tr:512], start=first, stop=last,
                            )
                        # normalize: fast-recip of denom row, broadcast via
                        # selector matmul, multiply into yT
                        for hh, psy in ((0, psyA), (1, psyB)):
                            # 1/x as exp(-ln(x)) on the ACT engine; both funcs
                            # live in one table set, and the second activation
                            # writes the bf16 broadcast operand directly
                            nc.scalar.activation(
                                rrecF[hh][64:65, :], psy[64:65, :], LNF
                            )
                            nc.scalar.activation(
                                rrec16[hh][64:65, :], rrecF[hh][64:65, :],
                                EXPF, scale=-1.0,
                            )
                            pbc = sps.tile([64, 512], F32, tag="sA", name="pbc")
                            nc.tensor.matmul(
                                pbc[:], e64_sb[:], rrec16[hh][:],
                                start=True, stop=True,
                            )
                            bc = ap_.tile([64, 512], R32, tag="bc")
                            nc.vector.tensor_copy(bc[:], pbc[:])
                            if hh == 0:
                                nc.vector.tensor_tensor(
                                    yT[p][0:64, lo:hi], psy[0:64, :], bc[:], MULT
                                )
                            else:
                                tb = ap_.tile([64, 512], R32, tag="tb")
                                nc.vector.tensor_tensor(
                                    tb[:], psy[0:64, :], bc[:], MULT
                                )
                                nc.sync.dma_start(yT[p][64:128, lo:hi], tb[:])

            # ---- phase OUT: output projection (partial; host sums over cores)
            with (
                tc.tile_pool(name="oph", bufs=3) as op_,
                tc.tile_pool(name="ow", bufs=1) as ow,
                tc.tile_pool(name="ops", bufs=4, space="PSUM") as ops,
            ):
                wo_sb = ow.tile([128, NP, C], R32)
                nc.sync.dma_start(wo_sb[:], wo_d)
                for mt in range(TT):
                    mlo, mhi = mt * 128, (mt + 1) * 128
                    for cc in range(2):
                        clo, chi = cc * 512, (cc + 1) * 512
                        ps = ops.tile([128, 512], F32, tag="po")
                        for r in range(NP):
                            nc.tensor.matmul(
                                ps[:], yT[r][:, mlo:mhi], wo_sb[:, r, clo:chi],
                                start=(r == 0), stop=(r == NP - 1),
                            )
                        ob = op_.tile([128, 512], F32, tag="ob")
                        nc.scalar.copy(ob[:], ps[:])
                        nc.sync.dma_start(out_d[mlo:mhi, clo:chi], ob[:])

    nc.compile()
    return nc


def _host_tables():
    half = D // 2
    freq = np.exp(-math.log(10000.0) * np.arange(half) / half).astype(np.float64)
    ang = np.arange(T, dtype=np.float64)[None, :] * freq[:, None]  # [32, T]
    cos32 = np.cos(ang).astype(np.float32)
    sin32 = np.sin(ang).astype(np.float32)
    cosT = np.tile(cos32, (4, 1))                                   # [128, T]
    sinT = np.concatenate([sin32, -sin32, sin32, -sin32], axis=0)   # [128, T]
    psw = np.zeros((128, 128), np.float32)
    psw[np.arange(128) ^ 32, np.arange(128)] = 1.0
    import ml_dtypes
    e64 = np.zeros((128, 64), ml_dtypes.bfloat16)
    e64[64, :] = 1.0
    kk = np.arange(128)[:, None]
    jj = np.arange(128)[None, :]
    msk = (jj >= kk).astype(np.float32)  # [128,128] diagonal band mask
    return cosT, sinT, psw, e64, msk


def _pack_weights(w_qkv, w_out, hg):
    lo, hi = hg * HL, (hg + 1) * HL
    wqf = w_qkv[:, 0:C].reshape(C, H, D)[:, lo:hi]       # [C, 8, D]
    wkf = w_qkv[:, C : 2 * C].reshape(C, H, D)[:, lo:hi]
    wvf = w_qkv[:, 2 * C : 3 * C].reshape(C, H, D)[:, lo:hi]

    def pack_qk(w):
        a = w.reshape(KT, 128, NP, 2, D)
        return np.ascontiguousarray(
            a.transpose(1, 2, 0, 3, 4).reshape(128, NP, KT, 128)
        )

    wq = pack_qk(wqf)
    wk = pack_qk(wkf)
    wv = np.ascontiguousarray(
        wvf.reshape(KT, 128, HL * D).transpose(1, 0, 2)
    )
    wo_l = w_out.reshape(H, D, C)[lo:hi].reshape(NP, 128, C)
    wo = np.ascontiguousarray(wo_l.transpose(1, 0, 2))
    return wq, wk, wv, wo


def _build_in_maps(x, w_qkv, w_out):
    x = np.asarray(x, dtype=np.float32)
    w_qkv = np.asarray(w_qkv, dtype=np.float32)
    w_out = np.asarray(w_out, dtype=np.float32)
    cosT, sinT, psw, e64, msk = _host_tables()
    packs = [_pack_weights(w_qkv, w_out, hg) for hg in range(2)]
    xTs = [
        np.ascontiguousarray(
            x[b].T.reshape(KT, 128, T).transpose(1, 0, 2)
        )
        for b in range(B)
    ]
    in_maps = []
    for c in range(8):
        b, hg = c // 2, c % 2
        wq, wk, wv, wo = packs[hg]
        in_maps.append(
            {
                "xT": xTs[b], "wq": wq, "wk": wk, "wv": wv, "wo": wo,
                "cosT": cosT, "sinT": sinT, "psw": psw, "e64": e64,
                "msk": msk,
            }
        )
    return in_maps


def kernel(x, w_qkv, w_out):
    if "nc" not in _CACHE:
        _CACHE["nc"] = _build_nc()
    nc = _CACHE["nc"]
    in_maps = _build_in_maps(x, w_qkv, w_out)
    res = run_bass_kernel_spmd(nc, in_maps, core_ids=list(range(8)))
    outs = [res.results[c]["out"] for c in range(8)]
    y = np.stack([outs[2 * b] + outs[2 * b + 1] for b in range(B)], axis=0)
    return y.astype(np.float32)


# revision 7
# speedup vs baseline: 1.5556x; 1.1731x over previous
"""Causal self-attention (RoPE) Trainium2 Bass kernel.

Sharding: 8 cores = 4 batches x 2 head-groups. Core c handles batch c//2 and
heads (c%2)*8 .. (c%2)*8+7. Each core computes its QKV projection slice, RoPE,
causal flash-style attention in transposed layout, and a partial output
projection; the host sums the two partial projections per batch.

Everything upstream of PSUM runs in bf16 (inputs are rounded host-side);
accumulation stays fp32, so the end-to-end error is ~2e-3 against the fp32
reference. Attention is computed transposed (s^T = k q^T) so softmax
denominators come from an appended ones-column in the value matrix and
attention output feeds the output projection as lhsT with no transposes.

The kernel is a single fused loop over 512-query chunks: each iteration
projects v/q/k for the chunk (RoPE fused), runs causal attention for all four
head pairs against every key chunk so far, and immediately emits the output
projection for the finished rows. That keeps the tensor engine busy through
the ACT-heavy attention phase. Causal tiles are trimmed to their exact valid
width; masking touches only the 128-wide diagonal band; softmax reciprocals
run as exp(-ln(x)) on the ACT engine over both heads at once.
"""

import math
import numpy as np
from contextlib import ExitStack

import concourse.bass as bass
import concourse.tile as tile
from concourse import bacc, mybir
from concourse.bass_utils import run_bass_kernel_spmd

F32 = mybir.dt.float32
BF16 = mybir.dt.bfloat16
EXPF = mybir.ActivationFunctionType.Exp
LNF = mybir.ActivationFunctionType.Ln
MULT = mybir.AluOpType.mult
ADD = mybir.AluOpType.add

B, T, C, H, D = 4, 2048, 1024, 16, 64
HL = 8            # local heads per core
NP = HL // 2      # head pairs per core
KT = C // 128     # contraction tiles for projections
TT = T // 128     # 128-row tiles of T
QC = T // 512     # 512-col chunks of T
SCALE = 1.0 / math.sqrt(D)

_CACHE = {}


def _build_nc():
    nc = bacc.Bacc("TRN2", debug=False, num_devices=8)

    xT_d = nc.dram_tensor("xT", [128, KT, T], BF16, kind="ExternalInput").ap()
    wq_d = nc.dram_tensor("wq", [128, NP, KT, 128], BF16, kind="ExternalInput").ap()
    wk_d = nc.dram_tensor("wk", [128, NP, KT, 128], BF16, kind="ExternalInput").ap()
    wv_d = nc.dram_tensor("wv", [128, KT, 512], BF16, kind="ExternalInput").ap()
    wo_d = nc.dram_tensor("wo", [128, NP, C], BF16, kind="ExternalInput").ap()
    cos_d = nc.dram_tensor("cosT", [128, T], F32, kind="ExternalInput").ap()
    sin_d = nc.dram_tensor("sinT", [128, T], F32, kind="ExternalInput").ap()
    psw_d = nc.dram_tensor("psw", [128, 128], BF16, kind="ExternalInput").ap()
    e64_d = nc.dram_tensor("e64", [128, 64], BF16, kind="ExternalInput").ap()
    msk_d = nc.dram_tensor("msk", [128, 128], BF16, kind="ExternalInput").ap()
    out_d = nc.dram_tensor("out", [T, C], F32, kind="ExternalOutput").ap()

    with tile.TileContext(nc) as tc:
        with ExitStack() as ctx:
            pers = ctx.enter_context(tc.tile_pool(name="pers", bufs=1))
            vext = pers.tile([128, TT, HL, D + 1], BF16)
            qkT = {}
            for p in range(NP):
                for s in "qk":
                    qkT[(p, s)] = pers.tile([128, T], BF16, name=f"qkT_{p}_{s}")
            yT = [pers.tile([128, T], BF16, name=f"yT_{r}") for r in range(NP)]
            nc.gpsimd.memset(vext[:, :, :, D], 1.0)

            wv_sb = pers.tile([128, KT, 512], BF16)
            psw_sb = pers.tile([128, 128], BF16)
            wq_sb = pers.tile([128, NP, KT, 128], BF16)
            wk_sb = pers.tile([128, NP, KT, 128], BF16)
            cos_sb = pers.tile([128, T], F32)
            sin_sb = pers.tile([128, T], F32)
            e64_sb = pers.tile([128, 64], BF16)
            msk_sb = pers.tile([128, 128], BF16)
            wo_sb = pers.tile([128, NP, C], BF16)
            rrecF = [
                pers.tile([128, 1024], F32, name=f"rrecF{i}") for i in range(2)
            ]
            rrec16 = [
                pers.tile([128, 1024], BF16, name=f"rrec16_{i}")
                for i in range(2)
            ]
            # first-needed first: the value projection only waits on wv
            nc.sync.dma_start(wv_sb[:], wv_d)
            nc.sync.dma_start(psw_sb[:], psw_d)
            nc.sync.dma_start(wq_sb[:], wq_d)
            nc.sync.dma_start(wk_sb[:], wk_d)
            nc.sync.dma_start(cos_sb[:], cos_d)
            nc.sync.dma_start(sin_sb[:], sin_d)
            nc.sync.dma_start(e64_sb[:], e64_d)
            nc.sync.dma_start(msk_sb[:], msk_d)
            nc.sync.dma_start(wo_sb[:], wo_d)
            for rr in rrec16:
                nc.gpsimd.memset(rr[:], 0.0)

            with (
                tc.tile_pool(name="sbw", bufs=2) as sp,
                tc.tile_pool(name="attp", bufs=4) as ap_,
                tc.tile_pool(name="oph", bufs=3) as op_,
                tc.tile_pool(name="pps", bufs=2, space="PSUM") as pps,
                tc.tile_pool(name="sps", bufs=2, space="PSUM") as sps,
                tc.tile_pool(name="yps", bufs=1, space="PSUM") as yps,
            ):
                nrm = 0
                for qc in range(QC):
                    lo, hi = qc * 512, (qc + 1) * 512
                    xc = sp.tile([128, KT, 512], BF16, tag="xc")
                    nc.sync.dma_start(xc[:], xT_d[:, :, lo:hi])
                    # ---- value projection for the 4 row-tiles of this chunk;
                    # the copy runs on the scalar engine
                    for ts in range(4):
                        tt = qc * 4 + ts
                        pv = pps.tile([128, 512], F32, tag="pp", name="pv")
                        for kt in range(KT):
                            nc.tensor.matmul(
                                pv[:],
                                xc[:, kt, ts * 128 : (ts + 1) * 128],
                                wv_sb[:, kt],
                                start=(kt == 0), stop=(kt == KT - 1),
                            )
                        nc.scalar.copy(vext[:, tt, :, 0:D], pv[:])
                    # ---- q/k projection + RoPE for this chunk
                    for p in range(NP):
                        for w_sb, key in ((wq_sb, "q"), (wk_sb, "k")):
                            dst = qkT[(p, key)][:, lo:hi]
                            ps = pps.tile([128, 512], F32, tag="pp", name="ps")
                            for kt in range(KT):
                                nc.tensor.matmul(
                                    ps[:], w_sb[:, p, kt], xc[:, kt],
                                    start=(kt == 0), stop=(kt == KT - 1),
                                )
                            nc.vector.tensor_tensor(
                                dst, ps[:], cos_sb[:, lo:hi], MULT
                            )
                            u = sp.tile([128, 512], BF16, tag="u")
                            nc.vector.tensor_tensor(
                                u[:], ps[:], sin_sb[:, lo:hi], MULT
                            )
                            pr = pps.tile([128, 512], F32, tag="pp", name="pr")
                            nc.tensor.matmul(
                                pr[:], psw_sb[:], u[:], start=True, stop=True
                            )
                            nc.vector.tensor_tensor(dst, pr[:], dst, ADD)
                    # ---- causal attention: queries of this chunk vs all keys
                    # so far, per head pair, transposed
                    nkt = (qc + 1) * 4
                    for p in range(NP):
                        qTt = qkT[(p, "q")]
                        kTt = qkT[(p, "k")]
                        psy = yps.tile([65, 1024], F32, tag="yab")
                        for kt in range(nkt):
                            first, last = kt == 0, kt == nkt - 1
                            klo, khi = kt * 128, (kt + 1) * 128
                            off = klo - lo
                            # trim to the exact causally-valid query range
                            tr = max(off, 0)
                            ps2 = sps.tile([128, 1024], F32, tag="sA")
                            p3 = ps2[:].rearrange("p (h n) -> p h n", h=2)
                            nc.tensor.matmul(
                                ps2[:, tr:512],
                                kTt[0:64, klo:khi], qTt[0:64, lo + tr : hi],
                                start=True, stop=True,
                            )
                            nc.tensor.matmul(
                                ps2[:, 512 + tr : 1024],
                                kTt[64:128, klo:khi], qTt[64:128, lo + tr : hi],
                                start=True, stop=True,
                            )
                            aAB = ap_.tile([128, 1024], BF16, tag="aA")
                            a3 = aAB[:].rearrange("p (h n) -> p h n", h=2)
                            aA = aAB[:, 0:512]
                            aB = aAB[:, 512:1024]
                            nc.scalar.activation(
                                a3[:, :, tr:512], p3[:, :, tr:512],
                                EXPF, scale=SCALE,
                            )
                            if off >= 0:
                                # only the 128-wide diagonal band needs the
                                # triangular mask; columns past it are fully
                                # visible
                                nc.vector.tensor_tensor(
                                    aA[:, tr : tr + 128], aA[:, tr : tr + 128],
                                    msk_sb[:], MULT,
                                )
                                nc.vector.tensor_tensor(
                                    aB[:, tr : tr + 128], aB[:, tr : tr + 128],
                                    msk_sb[:], MULT,
                                )
                            nc.tensor.matmul(
                                psy[:, tr:512], vext[:, kt, 2 * p, :],
                                aA[:, tr:512], start=first, stop=last,
                            )
                            nc.tensor.matmul(
                                psy[:, 512 + tr : 1024],
                                vext[:, kt, 2 * p + 1, :],
                                aB[:, tr:512], start=first, stop=last,
                            )
                        # normalize both heads at once: 1/denom as exp(-ln(x))
                        # on the ACT engine, broadcast via bf16 selector
                        # matmul, multiply into yT
                        rF, r16 = rrecF[nrm % 2], rrec16[nrm % 2]
                        nrm += 1
                        nc.scalar.activation(rF[64:65, :], psy[64:65, :], LNF)
                        nc.scalar.activation(
                            r16[64:65, :], rF[64:65, :], EXPF, scale=-1.0
                        )
                        pbc = sps.tile([64, 1024], F32, tag="sA", name="pbc")
                        nc.tensor.matmul(
                            pbc[:, 0:512], e64_sb[:], r16[:, 0:512],
                            start=True, stop=True,
                        )
                        nc.tensor.matmul(
                            pbc[:, 512:1024], e64_sb[:], r16[:, 512:1024],
                            start=True, stop=True,
                        )
                        bc = ap_.tile([64, 1024], BF16, tag="bc")
                        nc.vector.tensor_copy(bc[:], pbc[:])
                        nc.vector.tensor_tensor(
                            yT[p][0:64, lo:hi], psy[0:64, 0:512],
                            bc[:, 0:512], MULT,
                        )
                        tb = ap_.tile([64, 512], BF16, tag="tb")
                        nc.vector.tensor_tensor(
                            tb[:], psy[0:64, 512:1024], bc[:, 512:1024], MULT
                        )
                        nc.sync.dma_start(yT[p][64:128, lo:hi], tb[:])
                    # ---- output projection for the finished rows of this
                    # chunk (partial over heads; host sums the two cores)
                    for ts in range(4):
                        mlo, mhi = (qc * 4 + ts) * 128, (qc * 4 + ts + 1) * 128
                        for cc in range(2):
                            clo, chi = cc * 512, (cc + 1) * 512
                            po = sps.tile([128, 512], F32, tag="sA", name="po")
                            for r in range(NP):
                                nc.tensor.matmul(
                                    po[:], yT[r][:, mlo:mhi],
                                    wo_sb[:, r, clo:chi],
                                    start=(r == 0), stop=(r == NP - 1),
                                )
                            ob = op_.tile([128, 512], F32, tag="ob")
                            nc.scalar.copy(ob[:], po[:])
                            nc.sync.dma_start(out_d[mlo:mhi, clo:chi], ob[:])

    nc.compile()
    return nc


def _host_tables():
    import ml_dtypes

    half = D // 2
    freq = np.exp(-math.log(10000.0) * np.arange(half) / half).astype(np.float64)
    ang = np.arange(T, dtype=np.float64)[None, :] * freq[:, None]  # [32, T]
    cos32 = np.cos(ang).astype(np.float32)
    sin32 = np.sin(ang).astype(np.float32)
    cosT = np.tile(cos32, (4, 1))                                   # [128, T]
    sinT = np.concatenate([sin32, -sin32, sin32, -sin32], axis=0)   # [128, T]
    psw = np.zeros((128, 128), ml_dtypes.bfloat16)
    psw[np.arange(128) ^ 32, np.arange(128)] = 1.0
    e64 = np.zeros((128, 64), ml_dtypes.bfloat16)
    e64[64, :] = 1.0
    kk = np.arange(128)[:, None]
    jj = np.arange(128)[None, :]
    msk = (jj >= kk).astype(ml_dtypes.bfloat16)  # [128,128] diag band mask
    return cosT, sinT, psw, e64, msk


def _pack_weights(w_qkv, w_out, hg):
    import ml_dtypes

    lo, hi = hg * HL, (hg + 1) * HL
    wqf = w_qkv[:, 0:C].reshape(C, H, D)[:, lo:hi]       # [C, 8, D]
    wkf = w_qkv[:, C : 2 * C].reshape(C, H, D)[:, lo:hi]
    wvf = w_qkv[:, 2 * C : 3 * C].reshape(C, H, D)[:, lo:hi]

    def pack_qk(w):
        a = w.reshape(KT, 128, NP, 2, D)
        return np.ascontiguousarray(
            a.transpose(1, 2, 0, 3, 4).reshape(128, NP, KT, 128)
        ).astype(ml_dtypes.bfloat16)

    wq = pack_qk(wqf)
    wk = pack_qk(wkf)
    wv = np.ascontiguousarray(
        wvf.reshape(KT, 128, HL * D).transpose(1, 0, 2)
    ).astype(ml_dtypes.bfloat16)
    wo_l = w_out.reshape(H, D, C)[lo:hi].reshape(NP, 128, C)
    wo = np.ascontiguousarray(wo_l.transpose(1, 0, 2)).astype(
        ml_dtypes.bfloat16
    )
    return wq, wk, wv, wo


def _build_in_maps(x, w_qkv, w_out):
    import ml_dtypes

    x = np.asarray(x, dtype=np.float32)
    w_qkv = np.asarray(w_qkv, dtype=np.float32)
    w_out = np.asarray(w_out, dtype=np.float32)
    cosT, sinT, psw, e64, msk = _host_tables()
    packs = [_pack_weights(w_qkv, w_out, hg) for hg in range(2)]
    xTs = [
        np.ascontiguousarray(
            x[b].T.reshape(KT, 128, T).transpose(1, 0, 2)
        ).astype(ml_dtypes.bfloat16)
        for b in range(B)
    ]
    in_maps = []
    for c in range(8):
        b, hg = c // 2, c % 2
        wq, wk, wv, wo = packs[hg]
        in_maps.append(
            {
                "xT": xTs[b], "wq": wq, "wk": wk, "wv": wv, "wo": wo,
                "cosT": cosT, "sinT": sinT, "psw": psw, "e64": e64,
                "msk": msk,
            }
        )
    return in_maps


def kernel(x, w_qkv, w_out):
    if "nc" not in _CACHE:
        _CACHE["nc"] = _build_nc()
    nc = _CACHE["nc"]
    in_maps = _build_in_maps(x, w_qkv, w_out)
    res = run_bass_kernel_spmd(nc, in_maps, core_ids=list(range(8)))
    outs = [res.results[c]["out"] for c in range(8)]
    y = np.stack([outs[2 * b] + outs[2 * b + 1] for b in range(B)], axis=0)
    return y.astype(np.float32)


# revision 18
# speedup vs baseline: 1.6217x; 1.0425x over previous
"""Causal self-attention (RoPE) Trainium2 Bass kernel.

Sharding: 8 cores = 4 batches x 2 head-groups. Core c handles batch c//2 and
heads (c%2)*8 .. (c%2)*8+7. Each core computes its QKV projection slice, RoPE,
causal flash-style attention in transposed layout, and a partial output
projection; the host sums the two partial projections per batch.

Everything upstream of PSUM runs in bf16 (inputs are rounded host-side);
accumulation stays fp32, so the end-to-end error is ~2e-3 against the fp32
reference. Attention is computed transposed (s^T = k q^T) so softmax
denominators come from an appended ones-column in the value matrix and
attention output feeds the output projection as lhsT with no transposes.

The kernel is a single fused loop over 512-query chunks: each iteration
projects v/q/k for the chunk (RoPE fused), runs causal attention for all four
head pairs against every key chunk so far, and immediately emits the output
projection for the finished rows. That keeps the tensor engine busy through
the ACT-heavy attention phase. Causal tiles are trimmed to their exact valid
width; masking touches only the 128-wide diagonal band; softmax reciprocals
run as exp(-ln(x)) on the ACT engine over both heads at once.
"""

import math
import numpy as np
from contextlib import ExitStack

import concourse.bass as bass
import concourse.tile as tile
from concourse import bacc, mybir
from concourse.bass_utils import run_bass_kernel_spmd

F32 = mybir.dt.float32
BF16 = mybir.dt.bfloat16
EXPF = mybir.ActivationFunctionType.Exp
LNF = mybir.ActivationFunctionType.Ln
MULT = mybir.AluOpType.mult
ADD = mybir.AluOpType.add

B, T, C, H, D = 4, 2048, 1024, 16, 64
HL = 8            # local heads per core
NP = HL // 2      # head pairs per core
KT = C // 128     # contraction tiles for projections
TT = T // 128     # 128-row tiles of T
QC = T // 512     # 512-col chunks of T
SCALE = 1.0 / math.sqrt(D)

_CACHE = {}


def _build_nc():
    nc = bacc.Bacc("TRN2", debug=False, num_devices=8)

    xT_d = nc.dram_tensor("xT", [QC, 128, KT, 512], BF16, kind="ExternalInput").ap()
    wq_d = nc.dram_tensor("wq", [128, NP, KT, 128], BF16, kind="ExternalInput").ap()
    wk_d = nc.dram_tensor("wk", [128, NP, KT, 128], BF16, kind="ExternalInput").ap()
    wv_d = nc.dram_tensor("wv", [128, KT, 512], BF16, kind="ExternalInput").ap()
    wo_d = nc.dram_tensor("wo", [128, NP, C], BF16, kind="ExternalInput").ap()
    cos_d = nc.dram_tensor("cosT", [128, T], F32, kind="ExternalInput").ap()
    sin_d = nc.dram_tensor("sinT", [128, T], F32, kind="ExternalInput").ap()
    psw_d = nc.dram_tensor("psw", [128, 128], BF16, kind="ExternalInput").ap()
    e4_d = nc.dram_tensor("e4", [4, NP, 64], BF16, kind="ExternalInput").ap()
    msk_d = nc.dram_tensor("msk", [128, 128], BF16, kind="ExternalInput").ap()
    out_d = nc.dram_tensor("out", [T, C], F32, kind="ExternalOutput").ap()

    with tile.TileContext(nc) as tc:
        with ExitStack() as ctx:
            pers = ctx.enter_context(tc.tile_pool(name="pers", bufs=1))
            vext = pers.tile([128, TT, HL, D + 1], BF16)
            qkT = {}
            for p in range(NP):
                for s in "qk":
                    qkT[(p, s)] = pers.tile([128, T], BF16, name=f"qkT_{p}_{s}")
            yT = [pers.tile([128, T], BF16, name=f"yT_{r}") for r in range(NP)]
            nc.gpsimd.memset(vext[:, :, :, D], 1.0)

            wv_sb = pers.tile([128, KT, 512], BF16)
            psw_sb = pers.tile([128, 128], BF16)
            wq_sb = pers.tile([128, NP, KT, 128], BF16)
            wk_sb = pers.tile([128, NP, KT, 128], BF16)
            cos_sb = pers.tile([128, T], F32)
            sin_sb = pers.tile([128, T], F32)
            e4_sb = pers.tile([4, NP, 64], BF16)
            msk_sb = pers.tile([128, 128], BF16)
            wo_sb = pers.tile([128, NP, C], BF16)
            stg = [pers.tile([4, 1024], F32, name=f"stg{i}") for i in range(2)]
            stgL = [
                pers.tile([4, 1024], F32, name=f"stgL{i}") for i in range(2)
            ]
            r16s = [
                pers.tile([4, 1024], BF16, name=f"r16s{i}") for i in range(2)
            ]
            # first-needed first, split per contraction tile: the first
            # V-projection matmul only waits on wv[:, 0] and xc0[:, 0]
            for kt in range(KT):
                nc.sync.dma_start(wv_sb[:, kt], wv_d[:, kt])
            nc.sync.dma_start(psw_sb[:], psw_d)
            nc.sync.dma_start(wk_sb[:], wk_d)
            nc.sync.dma_start(cos_sb[:], cos_d)
            nc.sync.dma_start(sin_sb[:], sin_d)
            nc.sync.dma_start(e4_sb[:], e4_d)
            nc.sync.dma_start(msk_sb[:], msk_d)
            nc.sync.dma_start(wo_sb[:], wo_d)

            with (
                tc.tile_pool(name="sbw", bufs=2) as sp,
                tc.tile_pool(name="attp", bufs=4) as ap_,
                tc.tile_pool(name="oph", bufs=3) as op_,
                tc.tile_pool(name="pps", bufs=2, space="PSUM") as pps,
                tc.tile_pool(name="sps", bufs=2, space="PSUM") as sps,
                tc.tile_pool(name="yps", bufs=1, space="PSUM") as yps,
            ):
                nrm = 0
                xc0 = sp.tile([128, KT, 512], BF16, tag="xc", name="xc0")
                for kt in range(KT):
                    nc.sync.dma_start(xc0[:, kt], xT_d[0, :, kt])
                nc.sync.dma_start(wq_sb[:], wq_d)

                def load_chunk(qc):
                    xc = sp.tile([128, KT, 512], BF16, tag="xc", name="xc")
                    nc.sync.dma_start(xc[:], xT_d[qc])
                    return xc

                def proj_v(qc, xc):
                    # value projection for the 4 row-tiles of chunk qc; the
                    # copy runs on the vector engine
                    for ts in range(4):
                        tt = qc * 4 + ts
                        pv = pps.tile([128, 512], F32, tag="pp", name="pv")
                        for kt in range(KT):
                            nc.tensor.matmul(
                                pv[:],
                                xc[:, kt, ts * 128 : (ts + 1) * 128],
                                wv_sb[:, kt],
                                start=(kt == 0), stop=(kt == KT - 1),
                            )
                        nc.vector.tensor_copy(vext[:, tt, :, 0:D], pv[:])

                def proj_qk(qc, xc, p):
                    # q/k projection + RoPE for head pair p of chunk qc
                    lo, hi = qc * 512, (qc + 1) * 512
                    for w_sb, key in ((wq_sb, "q"), (wk_sb, "k")):
                        dst = qkT[(p, key)][:, lo:hi]
                        ps = pps.tile([128, 512], F32, tag="pp", name="ps")
                        for kt in range(KT):
                            nc.tensor.matmul(
                                ps[:], w_sb[:, p, kt], xc[:, kt],
                                start=(kt == 0), stop=(kt == KT - 1),
                            )
                        nc.vector.tensor_tensor(
                            dst, ps[:], cos_sb[:, lo:hi], MULT
                        )
                        u = sp.tile([128, 512], BF16, tag="u", name="u")
                        nc.vector.tensor_tensor(
                            u[:], ps[:], sin_sb[:, lo:hi], MULT
                        )
                        pr = pps.tile([128, 512], F32, tag="pp", name="pr")
                        nc.tensor.matmul(
                            pr[:], psw_sb[:], u[:], start=True, stop=True
                        )
                        nc.vector.tensor_tensor(dst, pr[:], dst, ADD)

                def out_proj(qc, ts):
                    # output projection for one 128-row tile (partial over
                    # heads; host sums the two cores)
                    mlo, mhi = (qc * 4 + ts) * 128, (qc * 4 + ts + 1) * 128
                    for cc in range(2):
                        clo, chi = cc * 512, (cc + 1) * 512
                        po = sps.tile([128, 512], F32, tag="sA", name="po")
                        for r in range(NP):
                            nc.tensor.matmul(
                                po[:], yT[r][:, mlo:mhi],
                                wo_sb[:, r, clo:chi],
                                start=(r == 0), stop=(r == NP - 1),
                            )
                        ob = op_.tile([128, 512], F32, tag="ob")
                        nc.vector.tensor_copy(ob[:], po[:])
                        nc.sync.dma_start(out_d[mlo:mhi, clo:chi], ob[:])

                def att_p(qc, p):
                    # causal attention for head pair p: queries of chunk qc
                    # vs all keys so far, transposed
                    nonlocal nrm
                    lo, hi = qc * 512, (qc + 1) * 512
                    nkt = (qc + 1) * 4
                    qTt = qkT[(p, "q")]
                    kTt = qkT[(p, "k")]
                    psy = yps.tile([65, 1024], F32, tag="yab", name="psy")
                    for kt in range(nkt):
                            first, last = kt == 0, kt == nkt - 1
                            klo, khi = kt * 128, (kt + 1) * 128
                            off = klo - lo
                            # trim to the exact causally-valid query range
                            tr = max(off, 0)
                            ps2 = sps.tile([128, 1024], F32, tag="sA")
                            p3 = ps2[:].rearrange("p (h n) -> p h n", h=2)
                            nc.tensor.matmul(
                                ps2[:, tr:512],
                                kTt[0:64, klo:khi], qTt[0:64, lo + tr : hi],
                                start=True, stop=True,
                            )
                            nc.tensor.matmul(
                                ps2[:, 512 + tr : 1024],
                                kTt[64:128, klo:khi], qTt[64:128, lo + tr : hi],
                                start=True, stop=True,
                            )
                            aAB = ap_.tile([128, 1024], BF16, tag="aA")
                            a3 = aAB[:].rearrange("p (h n) -> p h n", h=2)
                            aA = aAB[:, 0:512]
                            aB = aAB[:, 512:1024]
                            nc.scalar.activation(
                                a3[:, :, tr:512], p3[:, :, tr:512],
                                EXPF, scale=SCALE,
                            )
                            if off >= 0:
                                # only the 128-wide diagonal band needs the
                                # triangular mask; columns past it are fully
                                # visible
                                nc.vector.tensor_tensor(
                                    aA[:, tr : tr + 128], aA[:, tr : tr + 128],
                                    msk_sb[:], MULT,
                                )
                                nc.vector.tensor_tensor(
                                    aB[:, tr : tr + 128], aB[:, tr : tr + 128],
                                    msk_sb[:], MULT,
                                )
                            nc.tensor.matmul(
                                psy[:, tr:512], vext[:, kt, 2 * p, :],
                                aA[:, tr:512], start=first, stop=last,
                            )
                            nc.tensor.matmul(
                                psy[:, 512 + tr : 1024],
                                vext[:, kt, 2 * p + 1, :],
                                aB[:, tr:512], start=first, stop=last,
                            )
                        # normalize both heads at once: 1/denom as exp(-ln(x))
                        # on the ACT engine, broadcast via bf16 selector
                        # matmul, multiply into yT
                        rF, r16 = rrecF[nrm % 2], rrec16[nrm % 2]
                        nrm += 1
                        nc.scalar.activation(rF[64:65, :], psy[64:65, :], LNF)
                        nc.scalar.activation(
                            r16[64:65, :], rF[64:65, :], EXPF, scale=-1.0
                        )
                        pbc = sps.tile([64, 1024], F32, tag="sA", name="pbc")
                        nc.tensor.matmul(
                            pbc[:, 0:512], e64_sb[:], r16[:, 0:512],
                            start=True, stop=True,
                        )
                        nc.tensor.matmul(
                            pbc[:, 512:1024], e64_sb[:], r16[:, 512:1024],
                            start=True, stop=True,
                        )
                        bc = ap_.tile([64, 1024], BF16, tag="bc")
                        nc.vector.tensor_copy(bc[:], pbc[:])
                        nc.vector.tensor_tensor(
                            yT[p][0:64, lo:hi], psy[0:64, 0:512],
                            bc[:, 0:512], MULT,
                        )
                        tb = ap_.tile([64, 512], BF16, tag="tb")
                        nc.vector.tensor_tensor(
                            tb[:], psy[0:64, 512:1024], bc[:, 512:1024], MULT
                        )
                        nc.sync.dma_start(yT[p][64:128, lo:hi], tb[:])
                    # ---- output projection for the finished rows of this
                    # chunk (partial over heads; host sums the two cores)
                    for ts in range(4):
                        mlo, mhi = (qc * 4 + ts) * 128, (qc * 4 + ts + 1) * 128
                        for cc in range(2):
                            clo, chi = cc * 512, (cc + 1) * 512
                            po = sps.tile([128, 512], F32, tag="sA", name="po")
                            for r in range(NP):
                                nc.tensor.matmul(
                                    po[:], yT[r][:, mlo:mhi],
                                    wo_sb[:, r, clo:chi],
                                    start=(r == 0), stop=(r == NP - 1),
                                )
                            ob = op_.tile([128, 512], F32, tag="ob")
                            nc.scalar.copy(ob[:], po[:])
                            nc.sync.dma_start(out_d[mlo:mhi, clo:chi], ob[:])

    nc.compile()
    return nc


def _host_tables():
    import ml_dtypes

    half = D // 2
    freq = np.exp(-math.log(10000.0) * np.arange(half) / half).astype(np.float64)
    ang = np.arange(T, dtype=np.float64)[None, :] * freq[:, None]  # [32, T]
    cos32 = np.cos(ang).astype(np.float32)
    sin32 = np.sin(ang).astype(np.float32)
    cosT = np.tile(cos32, (4, 1))                                   # [128, T]
    sinT = np.concatenate([sin32, -sin32, sin32, -sin32], axis=0)   # [128, T]
    psw = np.zeros((128, 128), ml_dtypes.bfloat16)
    psw[np.arange(128) ^ 32, np.arange(128)] = 1.0
    e4 = np.zeros((4, 4, 64), ml_dtypes.bfloat16)
    for p in range(4):
        e4[p, p, :] = 1.0
    kk = np.arange(128)[:, None]
    jj = np.arange(128)[None, :]
    msk = (jj >= kk).astype(ml_dtypes.bfloat16)  # [128,128] diag band mask
    return cosT, sinT, psw, e4, msk


def _pack_weights(w_qkv, w_out, hg):
    import ml_dtypes

    lo, hi = hg * HL, (hg + 1) * HL
    wqf = w_qkv[:, 0:C].reshape(C, H, D)[:, lo:hi]       # [C, 8, D]
    wkf = w_qkv[:, C : 2 * C].reshape(C, H, D)[:, lo:hi]
    wvf = w_qkv[:, 2 * C : 3 * C].reshape(C, H, D)[:, lo:hi]

    def pack_qk(w):
        a = w.reshape(KT, 128, NP, 2, D)
        return np.ascontiguousarray(
            a.transpose(1, 2, 0, 3, 4).reshape(128, NP, KT, 128)
        ).astype(ml_dtypes.bfloat16)

    wq = pack_qk(wqf)
    wk = pack_qk(wkf)
    wv = np.ascontiguousarray(
        wvf.reshape(KT, 128, HL * D).transpose(1, 0, 2)
    ).astype(ml_dtypes.bfloat16)
    wo_l = w_out.reshape(H, D, C)[lo:hi].reshape(NP, 128, C)
    wo = np.ascontiguousarray(wo_l.transpose(1, 0, 2)).astype(
        ml_dtypes.bfloat16
    )
    return wq, wk, wv, wo


def _build_in_maps(x, w_qkv, w_out):
    import ml_dtypes

    x = np.asarray(x, dtype=np.float32)
    w_qkv = np.asarray(w_qkv, dtype=np.float32)
    w_out = np.asarray(w_out, dtype=np.float32)
    cosT, sinT, psw, e4, msk = _host_tables()
    packs = [_pack_weights(w_qkv, w_out, hg) for hg in range(2)]
    xTs = [
        np.ascontiguousarray(
            x[b].T.reshape(KT, 128, QC, 512).transpose(2, 1, 0, 3)
        ).astype(ml_dtypes.bfloat16)
        for b in range(B)
    ]
    in_maps = []
    for c in range(8):
        b, hg = c // 2, c % 2
        wq, wk, wv, wo = packs[hg]
        in_maps.append(
            {
                "xT": xTs[b], "wq": wq, "wk": wk, "wv": wv, "wo": wo,
                "cosT": cosT, "sinT": sinT, "psw": psw, "e4": e4,
                "msk": msk,
            }
        )
    return in_maps


def kernel(x, w_qkv, w_out):
    if "nc" not in _CACHE:
        _CACHE["nc"] = _build_nc()
    nc = _CACHE["nc"]
    in_maps = _build_in_maps(x, w_qkv, w_out)
    res = run_bass_kernel_spmd(nc, in_maps, core_ids=list(range(8)))
    outs = [res.results[c]["out"] for c in range(8)]
    y = np.stack([outs[2 * b] + outs[2 * b + 1] for b in range(B)], axis=0)
    return y.astype(np.float32)


# revision 19
# speedup vs baseline: 1.6797x; 1.0358x over previous
"""Causal self-attention (RoPE) Trainium2 Bass kernel.

Sharding: 8 cores = 4 batches x 2 head-groups. Core c handles batch c//2 and
heads (c%2)*8 .. (c%2)*8+7. Each core computes its QKV projection slice, RoPE,
causal flash-style attention in transposed layout, and a partial output
projection; the host sums the two partial projections per batch.

Everything upstream of PSUM runs in bf16 (inputs are rounded host-side);
accumulation stays fp32, so the end-to-end error is ~2e-3 against the fp32
reference. Attention is computed transposed (s^T = k q^T) so softmax
denominators come from an appended ones-column in the value matrix and
attention output feeds the output projection as lhsT with no transposes.

The kernel is a single fused loop over 512-query chunks: each iteration
projects v/q/k for the chunk (RoPE fused), runs causal attention for all four
head pairs against every key chunk so far, and immediately emits the output
projection for the finished rows. That keeps the tensor engine busy through
the ACT-heavy attention phase. Causal tiles are trimmed to their exact valid
width; masking touches only the 128-wide diagonal band; softmax reciprocals
run as exp(-ln(x)) on the ACT engine over both heads at once.
"""

import math
import numpy as np
from contextlib import ExitStack

import concourse.bass as bass
import concourse.tile as tile
from concourse import bacc, mybir
from concourse.bass_utils import run_bass_kernel_spmd

F32 = mybir.dt.float32
BF16 = mybir.dt.bfloat16
EXPF = mybir.ActivationFunctionType.Exp
LNF = mybir.ActivationFunctionType.Ln
MULT = mybir.AluOpType.mult
ADD = mybir.AluOpType.add

B, T, C, H, D = 4, 2048, 1024, 16, 64
HL = 8            # local heads per core
NP = HL // 2      # head pairs per core
KT = C // 128     # contraction tiles for projections
TT = T // 128     # 128-row tiles of T
QC = T // 512     # 512-col chunks of T
SCALE = 1.0 / math.sqrt(D)

_CACHE = {}


def _build_nc():
    nc = bacc.Bacc("TRN2", debug=False, num_devices=8)

    xT_d = nc.dram_tensor("xT", [QC, 128, KT, 512], BF16, kind="ExternalInput").ap()
    wq_d = nc.dram_tensor("wq", [128, NP, KT, 128], BF16, kind="ExternalInput").ap()
    wk_d = nc.dram_tensor("wk", [128, NP, KT, 128], BF16, kind="ExternalInput").ap()
    wv_d = nc.dram_tensor("wv", [128, KT, 512], BF16, kind="ExternalInput").ap()
    wo_d = nc.dram_tensor("wo", [128, NP, C], BF16, kind="ExternalInput").ap()
    cos_d = nc.dram_tensor("cosT", [128, T], F32, kind="ExternalInput").ap()
    sin_d = nc.dram_tensor("sinT", [128, T], F32, kind="ExternalInput").ap()
    psw_d = nc.dram_tensor("psw", [128, 128], BF16, kind="ExternalInput").ap()
    e4_d = nc.dram_tensor("e4", [4, NP, 64], BF16, kind="ExternalInput").ap()
    msk_d = nc.dram_tensor("msk", [128, 128], BF16, kind="ExternalInput").ap()
    out_d = nc.dram_tensor("out", [T, C], F32, kind="ExternalOutput").ap()

    with tile.TileContext(nc) as tc:
        with ExitStack() as ctx:
            pers = ctx.enter_context(tc.tile_pool(name="pers", bufs=1))
            vext = pers.tile([128, TT, HL, D + 1], BF16)
            qkT = {}
            for p in range(NP):
                for s in "qk":
                    qkT[(p, s)] = pers.tile([128, T], BF16, name=f"qkT_{p}_{s}")
            yT = [pers.tile([128, T], BF16, name=f"yT_{r}") for r in range(NP)]
            nc.gpsimd.memset(vext[:, :, :, D], 1.0)

            wv_sb = pers.tile([128, KT, 512], BF16)
            psw_sb = pers.tile([128, 128], BF16)
            wq_sb = pers.tile([128, NP, KT, 128], BF16)
            wk_sb = pers.tile([128, NP, KT, 128], BF16)
            cos_sb = pers.tile([128, T], F32)
            sin_sb = pers.tile([128, T], F32)
            e4_sb = pers.tile([4, NP, 64], BF16)
            msk_sb = pers.tile([128, 128], BF16)
            wo_sb = pers.tile([128, NP, C], BF16)
            stg = [pers.tile([4, 1024], F32, name=f"stg{i}") for i in range(2)]
            stgL = [
                pers.tile([4, 1024], F32, name=f"stgL{i}") for i in range(2)
            ]
            r16s = [
                pers.tile([4, 1024], BF16, name=f"r16s{i}") for i in range(2)
            ]
            # first-needed first: the value projection only waits on wv
            nc.sync.dma_start(wv_sb[:], wv_d)
            nc.sync.dma_start(psw_sb[:], psw_d)
            nc.sync.dma_start(wk_sb[:], wk_d)
            nc.sync.dma_start(cos_sb[:], cos_d)
            nc.sync.dma_start(sin_sb[:], sin_d)
            nc.sync.dma_start(e4_sb[:], e4_d)
            nc.sync.dma_start(msk_sb[:], msk_d)
            nc.sync.dma_start(wo_sb[:], wo_d)

            with (
                tc.tile_pool(name="sbw", bufs=2) as sp,
                tc.tile_pool(name="attp", bufs=4) as ap_,
                tc.tile_pool(name="oph", bufs=3) as op_,
                tc.tile_pool(name="pps", bufs=2, space="PSUM") as pps,
                tc.tile_pool(name="sps", bufs=2, space="PSUM") as sps,
                tc.tile_pool(name="yps", bufs=1, space="PSUM") as yps,
            ):
                nrm = 0
                xc0 = sp.tile([128, KT, 512], BF16, tag="xc", name="xc0")
                nc.sync.dma_start(xc0[:], xT_d[0])
                nc.sync.dma_start(wq_sb[:], wq_d)

                def load_chunk(qc):
                    xc = sp.tile([128, KT, 512], BF16, tag="xc", name="xc")
                    nc.sync.dma_start(xc[:], xT_d[qc])
                    return xc

                def proj_v(qc, xc):
                    # value projection for the 4 row-tiles of chunk qc; the
                    # copy runs on the vector engine
                    for ts in range(4):
                        tt = qc * 4 + ts
                        pv = pps.tile([128, 512], F32, tag="pp", name="pv")
                        for kt in range(KT):
                            nc.tensor.matmul(
                                pv[:],
                                xc[:, kt, ts * 128 : (ts + 1) * 128],
                                wv_sb[:, kt],
                                start=(kt == 0), stop=(kt == KT - 1),
                            )
                        nc.vector.tensor_copy(vext[:, tt, :, 0:D], pv[:])

                def proj_qk(qc, xc, p):
                    # q/k projection + RoPE for head pair p of chunk qc
                    lo, hi = qc * 512, (qc + 1) * 512
                    for w_sb, key in ((wq_sb, "q"), (wk_sb, "k")):
                        dst = qkT[(p, key)][:, lo:hi]
                        ps = pps.tile([128, 512], F32, tag="pp", name="ps")
                        for kt in range(KT):
                            nc.tensor.matmul(
                                ps[:], w_sb[:, p, kt], xc[:, kt],
                                start=(kt == 0), stop=(kt == KT - 1),
                            )
                        nc.vector.tensor_tensor(
                            dst, ps[:], cos_sb[:, lo:hi], MULT
                        )
                        u = sp.tile([128, 512], BF16, tag="u", name="u")
                        nc.vector.tensor_tensor(
                            u[:], ps[:], sin_sb[:, lo:hi], MULT
                        )
                        pr = pps.tile([128, 512], F32, tag="pp", name="pr")
                        nc.tensor.matmul(
                            pr[:], psw_sb[:], u[:], start=True, stop=True
                        )
                        nc.vector.tensor_tensor(dst, pr[:], dst, ADD)

                def out_proj(qc, ts):
                    # output projection for one 128-row tile (partial over
                    # heads; host sums the two cores)
                    mlo, mhi = (qc * 4 + ts) * 128, (qc * 4 + ts + 1) * 128
                    for cc in range(2):
                        clo, chi = cc * 512, (cc + 1) * 512
                        po = sps.tile([128, 512], F32, tag="sA", name="po")
                        for r in range(NP):
                            nc.tensor.matmul(
                                po[:], yT[r][:, mlo:mhi],
                                wo_sb[:, r, clo:chi],
                                start=(r == 0), stop=(r == NP - 1),
                            )
                        ob = op_.tile([128, 512], F32, tag="ob")
                        nc.vector.tensor_copy(ob[:], po[:])
                        nc.sync.dma_start(out_d[mlo:mhi, clo:chi], ob[:])

                def att_p(qc, p):
                    # causal attention for head pair p: queries of chunk qc
                    # vs all keys so far, transposed
                    nonlocal nrm
                    lo, hi = qc * 512, (qc + 1) * 512
                    nkt = (qc + 1) * 4
                    qTt = qkT[(p, "q")]
                    kTt = qkT[(p, "k")]
                    psy = yps.tile([65, 1024], F32, tag="yab", name="psy")
                    for kt in range(nkt):
                            first, last = kt == 0, kt == nkt - 1
                            klo, khi = kt * 128, (kt + 1) * 128
                            off = klo - lo
                            # trim to the exact causally-valid query range
                            tr = max(off, 0)
                            ps2 = sps.tile([128, 1024], F32, tag="sA")
                            p3 = ps2[:].rearrange("p (h n) -> p h n", h=2)
                            nc.tensor.matmul(
                                ps2[:, tr:512],
                                kTt[0:64, klo:khi], qTt[0:64, lo + tr : hi],
                                start=True, stop=True,
                            )
                            nc.tensor.matmul(
                                ps2[:, 512 + tr : 1024],
                                kTt[64:128, klo:khi], qTt[64:128, lo + tr : hi],
                                start=True, stop=True,
                            )
                            aAB = ap_.tile([128, 1024], BF16, tag="aA")
                            a3 = aAB[:].rearrange("p (h n) -> p h n", h=2)
                            aA = aAB[:, 0:512]
                            aB = aAB[:, 512:1024]
                            nc.scalar.activation(
                                a3[:, :, tr:512], p3[:, :, tr:512],
                                EXPF, scale=SCALE,
                            )
                            if off >= 0:
                                # only the 128-wide diagonal band needs the
                                # triangular mask; columns past it are fully
                                # visible
                                nc.vector.tensor_tensor(
                                    aA[:, tr : tr + 128], aA[:, tr : tr + 128],
                                    msk_sb[:], MULT,
                                )
                                nc.vector.tensor_tensor(
                                    aB[:, tr : tr + 128], aB[:, tr : tr + 128],
                                    msk_sb[:], MULT,
                                )
                            nc.tensor.matmul(
                                psy[:, tr:512], vext[:, kt, 2 * p, :],
                                aA[:, tr:512], start=first, stop=last,
                            )
                            nc.tensor.matmul(
                                psy[:, 512 + tr : 1024],
                                vext[:, kt, 2 * p + 1, :],
                                aB[:, tr:512], start=first, stop=last,
                            )
                        # normalize both heads at once: 1/denom as exp(-ln(x))
                        # on the ACT engine, broadcast via bf16 selector
                        # matmul, multiply into yT
                        rF, r16 = rrecF[nrm % 2], rrec16[nrm % 2]
                        nrm += 1
                        nc.scalar.activation(rF[64:65, :], psy[64:65, :], LNF)
                        nc.scalar.activation(
                            r16[64:65, :], rF[64:65, :], EXPF, scale=-1.0
                        )
                        pbc = sps.tile([64, 1024], F32, tag="sA", name="pbc")
                        nc.tensor.matmul(
                            pbc[:, 0:512], e64_sb[:], r16[:, 0:512],
                            start=True, stop=True,
                        )
                        nc.tensor.matmul(
                            pbc[:, 512:1024], e64_sb[:], r16[:, 512:1024],
                            start=True, stop=True,
                        )
                        bc = ap_.tile([64, 1024], BF16, tag="bc")
                        nc.vector.tensor_copy(bc[:], pbc[:])
                        nc.vector.tensor_tensor(
                            yT[p][0:64, lo:hi], psy[0:64, 0:512],
                            bc[:, 0:512], MULT,
                        )
                        tb = ap_.tile([64, 512], BF16, tag="tb")
                        nc.vector.tensor_tensor(
                            tb[:], psy[0:64, 512:1024], bc[:, 512:1024], MULT
                        )
                        nc.sync.dma_start(yT[p][64:128, lo:hi], tb[:])
                    # ---- output projection for the finished rows of this
                    # chunk (partial over heads; host sums the two cores)
                    for ts in range(4):
                        mlo, mhi = (qc * 4 + ts) * 128, (qc * 4 + ts + 1) * 128
                        for cc in range(2):
                            clo, chi = cc * 512, (cc + 1) * 512
                            po = sps.tile([128, 512], F32, tag="sA", name="po")
                            for r in range(NP):
                                nc.tensor.matmul(
                                    po[:], yT[r][:, mlo:mhi],
                                    wo_sb[:, r, clo:chi],
                                    start=(r == 0), stop=(r == NP - 1),
                                )
                            ob = op_.tile([128, 512], F32, tag="ob")
                            nc.scalar.copy(ob[:], po[:])
                            nc.sync.dma_start(out_d[mlo:mhi, clo:chi], ob[:])

    nc.compile()
    return nc


def _host_tables():
    import ml_dtypes

    half = D // 2
    freq = np.exp(-math.log(10000.0) * np.arange(half) / half).astype(np.float64)
    ang = np.arange(T, dtype=np.float64)[None, :] * freq[:, None]  # [32, T]
    cos32 = np.cos(ang).astype(np.float32)
    sin32 = np.sin(ang).astype(np.float32)
    cosT = np.tile(cos32, (4, 1))                                   # [128, T]
    sinT = np.concatenate([sin32, -sin32, sin32, -sin32], axis=0)   # [128, T]
    psw = np.zeros((128, 128), ml_dtypes.bfloat16)
    psw[np.arange(128) ^ 32, np.arange(128)] = 1.0
    e4 = np.zeros((4, 4, 64), ml_dtypes.bfloat16)
    for p in range(4):
        e4[p, p, :] = 1.0
    kk = np.arange(128)[:, None]
    jj = np.arange(128)[None, :]
    msk = (jj >= kk).astype(ml_dtypes.bfloat16)  # [128,128] diag band mask
    return cosT, sinT, psw, e4, msk


def _pack_weights(w_qkv, w_out, hg):
    import ml_dtypes

    lo, hi = hg * HL, (hg + 1) * HL
    wqf = w_qkv[:, 0:C].reshape(C, H, D)[:, lo:hi]       # [C, 8, D]
    wkf = w_qkv[:, C : 2 * C].reshape(C, H, D)[:, lo:hi]
    wvf = w_qkv[:, 2 * C : 3 * C].reshape(C, H, D)[:, lo:hi]

    def pack_qk(w):
        a = w.reshape(KT, 128, NP, 2, D)
        return np.ascontiguousarray(
            a.transpose(1, 2, 0, 3, 4).reshape(128, NP, KT, 128)
        ).astype(ml_dtypes.bfloat16)

    wq = pack_qk(wqf)
    wk = pack_qk(wkf)
    wv = np.ascontiguousarray(
        wvf.reshape(KT, 128, HL * D).transpose(1, 0, 2)
    ).astype(ml_dtypes.bfloat16)
    wo_l = w_out.reshape(H, D, C)[lo:hi].reshape(NP, 128, C)
    wo = np.ascontiguousarray(wo_l.transpose(1, 0, 2)).astype(
        ml_dtypes.bfloat16
    )
    return wq, wk, wv, wo


def _build_in_maps(x, w_qkv, w_out):
    import ml_dtypes

    x = np.asarray(x, dtype=np.float32)
    w_qkv = np.asarray(w_qkv, dtype=np.float32)
    w_out = np.asarray(w_out, dtype=np.float32)
    cosT, sinT, psw, e4, msk = _host_tables()
    packs = [_pack_weights(w_qkv, w_out, hg) for hg in range(2)]
    xTs = [
        np.ascontiguousarray(
            x[b].T.reshape(KT, 128, QC, 512).transpose(2, 1, 0, 3)
        ).astype(ml_dtypes.bfloat16)
        for b in range(B)
    ]
    in_maps = []
    for c in range(8):
        b, hg = c // 2, c % 2
        wq, wk, wv, wo = packs[hg]
        in_maps.append(
            {
                "xT": xTs[b], "wq": wq, "wk": wk, "wv": wv, "wo": wo,
                "cosT": cosT, "sinT": sinT, "psw": psw, "e4": e4,
                "msk": msk,
            }
        )
    return in_maps


def kernel(x, w_qkv, w_out):
    if "nc" not in _CACHE:
        _CACHE["nc"] = _build_nc()
    nc = _CACHE["nc"]
    in_maps = _build_in_maps(x, w_qkv, w_out)
    res = run_bass_kernel_spmd(nc, in_maps, core_ids=list(range(8)))
    outs = [res.results[c]["out"] for c in range(8)]
    y = np.stack([outs[2 * b] + outs[2 * b + 1] for b in range(B)], axis=0)
    return y.astype(np.float32)
